# revision 12
# baseline (speedup 1.0000x reference)
"""Trainium2 Bass kernel for ExpLambsMessageAggregator (gnn message passing).

Computes, per node n:
    t_last[n]   = max over events of node n of timestamp           (segment max)
    agg[n,k,d]  = sum_e m[e,d] * exp((t_e - t_last[n]) / lamb_k)   (segment sum)

Sharding: seg_ids are sorted, so the 65536 nodes are split into 8 ranges of
8192 nodes (one per NeuronCore); each core's events form a contiguous slice of
the event stream.  Within a core, nodes are grouped into 64 windows of 128
nodes; each window's events (contiguous) are padded to a common capacity C
(multiple of 128) so all cores run one identical SPMD program.

Device algorithm per 128-event tile of a window:
    A[e, k*128+n] = (iota[n] == local_id[e]) * exp(dt[e] / lamb_k)
        built with fused DVE tensor_scalar (is_equal, mult) ops
    psum[d, k*128+n] += sum_e m[e, d] * A[e, k*128+n]
        one fp32 matmul (lhsT = m tile, stationary; rhs = A, moving),
        accumulated over the window's tiles in one PSUM bank.
The [d,(k,n)] window result is evacuated by ScalarE and DMA'd out; the host
reorders to [n,k,d].  t_last is a pure gather (timestamps are globally
sorted, so each segment's last event holds its max); empty segments get -inf
to match jax.ops.segment_max's identity.

Raw bass (explicit semaphores): this toolchain's walrus allows at most ONE
semaphore wait attached to a compute instruction, so every cross-engine
dependency is expressed as a standalone wait_ge sequencer instruction.
"""

import os
import sys
from contextlib import ExitStack

import numpy as np

if "/opt/trn_rl_repo" not in sys.path:
    sys.path.insert(0, "/opt/trn_rl_repo")

import concourse.bass as bass
from concourse import mybir
from concourse.bass_utils import run_bass_kernel_spmd

N_NODES = 65536
N_CORES = 8
D = 128
K = 4
WN = 128  # nodes per window = one-hot width
F32 = mybir.dt.float32

# Filled in by kernel() on each call; test.py reads these for reporting.
last_run_info = {}


def _build_program(W, T, inv_lambs):
    """One core's program: W windows x (T tiles of 128 events each)."""
    C = T * 128
    nc = bass.Bass()
    m_in = nc.dram_tensor("m_pad", [W, C, D], F32, kind="ExternalInput")
    meta = nc.dram_tensor("meta", [W, 128, 2, T], F32, kind="ExternalInput")  # dt, lid
    out = nc.dram_tensor("out_dev", [W, D, K * WN], F32, kind="ExternalOutput")
    Exp = mybir.ActivationFunctionType.Exp

    with ExitStack() as ctx:
        ecm = ctx.enter_context
        m_sb = [ecm(nc.sbuf_tensor(f"m_sb{i}", [128, T, D], F32)) for i in range(2)]
        meta_sb = [
            ecm(nc.sbuf_tensor(f"meta_sb{i}", [128, 2, T], F32)) for i in range(2)
        ]
        wmat_sb = [
            ecm(nc.sbuf_tensor(f"wmat_sb{i}", [128, K, T], F32)) for i in range(2)
        ]
        a_sb = [ecm(nc.sbuf_tensor(f"a_sb{i}", [128, K * WN], F32)) for i in range(2)]
        stage_sb = [
            ecm(nc.sbuf_tensor(f"stage_sb{i}", [128, K * WN], F32)) for i in range(2)
        ]
        ps = [ecm(nc.psum_tensor(f"ps{i}", [128, K * WN], F32)) for i in range(2)]
        iota_i = ecm(nc.sbuf_tensor("iota_i", [128, WN], mybir.dt.int32))
        iota_f = ecm(nc.sbuf_tensor("iota_f", [128, WN], F32))

        s_m = ecm(nc.semaphore("s_m"))  # m tile DMAs done: 16*(w*T+t+1)
        s_meta = ecm(nc.semaphore("s_meta"))  # meta DMA done: 16*(w+1)
        s_w = ecm(nc.semaphore("s_w"))  # wmat ready: w+1
        s_a = ecm(nc.semaphore("s_a"))  # A tile ready: w*T+t+1
        s_mm = ecm(nc.semaphore("s_mm"))  # matmul done: w*T+t+1
        s_ev = ecm(nc.semaphore("s_ev"))  # evac done: w+1
        s_out = ecm(nc.semaphore("s_out"))  # out DMAs done: 32*(w+1)
        s_io = ecm(nc.semaphore("s_io"))  # iota ready

        with nc.Block() as block:

            @block.sync
            def _(sp):
                for w in range(W):
                    b = w % 2
                    if w >= 2:
                        # m_sb/meta_sb[b] reuse: matmuls of window w-2 done
                        # implies (transitively) all their readers are done.
                        sp.wait_ge(s_mm, (w - 1) * T)
                    src = m_in[w].rearrange("(t p) d -> p t d", p=128)
                    for t in range(T):
                        sp.dma_start(
                            out=m_sb[b][:, t, :], in_=src[:, t, :]
                        ).then_inc(s_m, 16)
                    sp.dma_start(out=meta_sb[b][:], in_=meta[w][:]).then_inc(
                        s_meta, 16
                    )
                    if w >= 1:
                        sp.wait_ge(s_ev, w)
                        v = w - 1
                        for h in range(2):  # split output across 2 queues
                            sp.dma_start(
                                out=out[v][h * 64 : (h + 1) * 64, :],
                                in_=stage_sb[v % 2][h * 64 : (h + 1) * 64, :],
                            ).then_inc(s_out, 16)
                sp.wait_ge(s_ev, W)
                for h in range(2):
                    sp.dma_start(
                        out=out[W - 1][h * 64 : (h + 1) * 64, :],
                        in_=stage_sb[(W - 1) % 2][h * 64 : (h + 1) * 64, :],
                    ).then_inc(s_out, 16)

            @block.gpsimd
            def _(pl):
                pl.iota(
                    iota_i[:], pattern=[[1, WN]], base=0, channel_multiplier=0
                ).then_inc(s_io, 1)

            @block.vector
            def _(v):
                v.wait_ge(s_io, 1)
                v.tensor_copy(iota_f[:], iota_i[:])
                for w in range(W):
                    b = w % 2
                    v.wait_ge(s_meta, 16 * (w + 1))
                    v.wait_ge(s_w, w + 1)
                    for t in range(T):
                        gt = w * T + t
                        if gt >= 2:
                            v.wait_ge(s_mm, gt - 1)  # a_sb[gt%2] consumed
                        for k in range(K):
                            ins = v.tensor_scalar(
                                a_sb[gt % 2][:, k * WN : (k + 1) * WN],
                                iota_f[:],
                                meta_sb[b][:, 1, t : t + 1],
                                wmat_sb[b][:, k, t : t + 1],
                                mybir.AluOpType.is_equal,
                                mybir.AluOpType.mult,
                            )
                        ins.then_inc(s_a, 1)

            @block.scalar
            def _(sc):
                def evac(vw):
                    sc.wait_ge(s_mm, (vw + 1) * T)
                    if vw >= 2:
                        sc.wait_ge(s_out, 32 * (vw - 1))  # stage_sb[vw%2] drained
                    sc.copy(out=stage_sb[vw % 2][:], in_=ps[vw % 2][:]).then_inc(
                        s_ev, 1
                    )

                for w in range(W):
                    b = w % 2
                    sc.wait_ge(s_meta, 16 * (w + 1))
                    if w >= 2:
                        sc.wait_ge(s_a, (w - 1) * T)  # wmat_sb[b] readers done
                    for k in range(K):
                        ins = sc.activation(
                            out=wmat_sb[b][:, k, :],
                            in_=meta_sb[b][:, 0, :],
                            func=Exp,
                            scale=float(inv_lambs[k]),
                        )
                    ins.then_inc(s_w, 1)
                    if w >= 1:
                        evac(w - 1)
                evac(W - 1)

            @block.tensor
            def _(te):
                for w in range(W):
                    b = w % 2
                    if w >= 2:
                        te.wait_ge(s_ev, w - 1)  # ps[b] evacuated
                    for t in range(T):
                        gt = w * T + t
                        te.wait_ge(s_m, 16 * (gt + 1))
                        te.wait_ge(s_a, gt + 1)
                        te.matmul(
                            ps[b][:],
                            m_sb[b][:, t, :],
                            a_sb[gt % 2][:],
                            start=(t == 0),
                            stop=(t == T - 1),
                        ).then_inc(s_mm, 1)

    return nc


def kernel(messages, timestamps, seg_ids, lambs):
    messages = np.ascontiguousarray(np.asarray(messages, dtype=np.float32))
    ts = np.asarray(timestamps, dtype=np.float32)
    seg = np.asarray(seg_ids).astype(np.int64)
    lambs_np = np.asarray(lambs, dtype=np.float32)
    n_win_total = N_NODES // WN  # 512
    win_per_core = n_win_total // N_CORES  # 64

    counts = np.bincount(seg, minlength=N_NODES)
    ends = np.cumsum(counts)
    starts = ends - counts
    last_idx = np.maximum(ends - 1, 0)
    t_last = np.where(counts > 0, ts[last_idx], -np.inf).astype(np.float32)

    # window event ranges (contiguous because seg is sorted)
    win_start = starts[0::WN]
    win_cnt = ends[WN - 1 :: WN] - win_start
    C = int(max(256, -(-int(win_cnt.max()) // 128) * 128))
    T = C // 128

    ar = np.arange(C)
    idx = win_start[:, None] + ar[None, :]
    valid = ar[None, :] < win_cnt[:, None]
    idxc = np.where(valid, idx, 0)

    m_pad = messages[idxc]  # [512, C, D]
    m_pad[~valid] = 0.0
    tl_fill = np.where(counts > 0, ts[last_idx], 0.0).astype(np.float32)
    seg_g = seg[idxc]
    bases = (np.arange(n_win_total, dtype=np.int64) * WN)[:, None]
    dt_h = np.where(valid, ts[idxc] - tl_fill[seg_g], 0.0).astype(np.float32)
    lid_h = np.where(valid, seg_g - bases, 0).astype(np.float32)
    # device layout [win, partition(=event%128), stream, tile]
    meta_all = np.stack(
        [
            dt_h.reshape(n_win_total, T, 128).transpose(0, 2, 1),
            lid_h.reshape(n_win_total, T, 128).transpose(0, 2, 1),
        ],
        axis=2,
    )
    meta_all = np.ascontiguousarray(meta_all)  # [512, 128, 2, T]

    inv_l = (1.0 / lambs_np.astype(np.float64)).astype(np.float32)
    nc = _build_program(win_per_core, T, inv_l)

    in_maps = [
        {
            "m_pad": m_pad[c * win_per_core : (c + 1) * win_per_core],
            "meta": meta_all[c * win_per_core : (c + 1) * win_per_core],
        }
        for c in range(N_CORES)
    ]

    trace = os.environ.get("KERNEL_TRACE", "0") == "1"
    kwargs = {}
    if trace:
        kwargs = dict(trace=True, trace_cores=[0])
    res = run_bass_kernel_spmd(nc, in_maps, core_ids=list(range(N_CORES)), **kwargs)

    last_run_info.clear()
    last_run_info.update(
        exec_time_ns=res.exec_time_ns,
        mean_exec_time_ns=res.mean_exec_time_ns,
        trace=res.instructions_and_trace[1] if res.instructions_and_trace else None,
    )

    n_loc = N_NODES // N_CORES
    agg = np.empty((N_NODES, K, D), np.float32)
    for c in range(N_CORES):
        r = res.results[c]["out_dev"].reshape(win_per_core, D, K, WN)
        agg[c * n_loc : (c + 1) * n_loc] = r.transpose(0, 3, 2, 1).reshape(n_loc, K, D)
    return agg, t_last


# revision 15
# speedup vs baseline: 1.3866x; 1.3866x over previous
"""Trainium2 Bass kernel for ExpLambsMessageAggregator (gnn message passing).

Computes, per node n:
    t_last[n]   = max over events of node n of timestamp           (segment max)
    agg[n,k,d]  = sum_e m[e,d] * exp((t_e - t_last[n]) / lamb_k)   (segment sum)

Sharding: seg_ids are sorted, so the 65536 nodes are split into 8 ranges of
8192 nodes (one per NeuronCore); each core's events form a contiguous slice
of the event stream.  Within a core, nodes are grouped into 256 windows of
32 nodes; each window's events (contiguous) are padded to a common capacity
C (multiple of 128) so all cores run one identical SPMD program.  Windows
are processed in groups of 4 to amortize DMA-issue and ScalarE overheads.

Device algorithm per 128-event tile of a window (events laid out
partition-major: partition p holds the window's events [p*T, (p+1)*T)):
    A[e, k*32+n] = (iota[n] == local_id[e]) * exp(dt[e] / lamb_k)
        ONE fused DVE scalar_tensor_tensor op (is_equal, mult) using
        stride-0 broadcast access patterns for iota (k-repeat) and the
        per-(e,k) weights (n-broadcast)
    psum[k*32+n, d] += sum_e A[e, k*32+n] * m[e, d]
        one fp32 matmul (lhsT = A, rhs = m tile), accumulating the
        window's T tiles into a quarter PSUM bank; 4 windows share a bank.
ScalarE computes the exp weights (scale=1/lamb fused) and evacuates each
group's [128,512] PSUM bank; the (k*32+n)-partition layout is scattered to
the [n,k,d] output by the store DMA's access pattern, so the host does no
reordering.  t_last is a pure gather (timestamps are globally sorted, so
each segment's last event holds its max); empty segments get -inf to match
jax.ops.segment_max's identity.

Raw bass with explicit semaphores: this toolchain's walrus allows at most
ONE semaphore wait attached to a compute instruction, so every cross-engine
dependency is a standalone wait_ge sequencer instruction, and semaphore
counts are arranged so each instruction needs at most one fresh wait.
"""

import os
import sys
from contextlib import ExitStack

import numpy as np

if "/opt/trn_rl_repo" not in sys.path:
    sys.path.insert(0, "/opt/trn_rl_repo")

import concourse.bass as bass
from concourse import mybir
from concourse.bass_utils import run_bass_kernel_spmd

N_NODES = 65536
N_CORES = 8
D = 128
K = 4
WN = 32  # nodes per window; K*WN = 128 one-hot columns (k,n)-packed
GRP = 4  # windows per group (share one PSUM bank / blob DMA / exp batch)
F32 = mybir.dt.float32
I32 = mybir.dt.int32

# Filled in by kernel() on each call; test.py reads these for reporting.
last_run_info = {}


def _bcast_ap(base, offset_elems, dims):
    """Hand-built AP on `base` (an AP) with explicit [step, count] dims."""
    return bass.AP(tensor=base.tensor, offset=base.offset + offset_elems, ap=dims)


def _build_program(n_grp, T, inv_lambs):
    """One core's program: n_grp groups x GRP windows x T tiles of 128 events."""
    R = T * D + 2 * T  # per-partition row: T m-vectors + T dt + T lid
    MM_G = GRP * T  # matmuls (= tiles) per group
    nc = bass.Bass()
    blob = nc.dram_tensor("blob", [n_grp, 128, GRP, R], F32, kind="ExternalInput")
    out = nc.dram_tensor("out_dev", [n_grp * GRP * WN, K * D], F32, kind="ExternalOutput")
    Exp = mybir.ActivationFunctionType.Exp

    with ExitStack() as ctx:
        ecm = ctx.enter_context
        blob_sb = [ecm(nc.sbuf_tensor(f"blob_sb{i}", [128, GRP, R], F32)) for i in range(2)]
        wmat_sb = [ecm(nc.sbuf_tensor(f"wmat_sb{i}", [128, K, GRP, T], F32)) for i in range(2)]
        a_sb = [ecm(nc.sbuf_tensor(f"a_sb{i}", [128, K * WN], F32)) for i in range(2)]
        stage_sb = [ecm(nc.sbuf_tensor(f"stage_sb{i}", [128, GRP, D], F32)) for i in range(2)]
        ps = [ecm(nc.psum_tensor(f"ps{i}", [128, GRP, D], F32)) for i in range(2)]
        iota_i = ecm(nc.sbuf_tensor("iota_i", [128, WN], I32))
        iota_f = ecm(nc.sbuf_tensor("iota_f", [128, WN], F32))

        s_in = ecm(nc.semaphore("s_in"))  # blob DMA done: 16*(g+1)
        s_w = ecm(nc.semaphore("s_w"))  # wmat ready: g+1
        s_a = ecm(nc.semaphore("s_a"))  # A ready: gt+1
        s_mm = ecm(nc.semaphore("s_mm"))  # matmul done: gt+1
        s_ev = ecm(nc.semaphore("s_ev"))  # evac done: g+1
        s_out = ecm(nc.semaphore("s_out"))  # out DMA done: 16*(g+1)
        s_io = ecm(nc.semaphore("s_io"))  # iota ready

        def out_dst_ap(g):
            # stage partition p = n*K + k maps to out row (g*GRP+w4)*WN + n,
            # col k*D + d: dst element offset = p*D + w4*WN*K*D + d (linear
            # in p because columns are (n,k)-packed).
            base = out[:]
            return _bcast_ap(
                base,
                g * GRP * WN * (K * D),
                [[D, WN * K], [WN * K * D, GRP], [1, D]],
            )

        with nc.Block() as block:

            @block.sync
            def _(sp):
                for g in range(n_grp):
                    b = g % 2
                    if g >= 2:
                        # blob_sb[b] reuse: group g-2 matmuls done implies all
                        # its readers (PE rhs, DVE lid, ACT dt) are done.
                        sp.wait_ge(s_mm, (g - 1) * MM_G)
                    sp.dma_start(out=blob_sb[b][:], in_=blob[g][:]).then_inc(s_in, 16)
                    if g >= 1:
                        sp.wait_ge(s_ev, g)
                        sp.dma_start(
                            out=out_dst_ap(g - 1), in_=stage_sb[(g - 1) % 2][:]
                        ).then_inc(s_out, 16)
                sp.wait_ge(s_ev, n_grp)
                sp.dma_start(
                    out=out_dst_ap(n_grp - 1), in_=stage_sb[(n_grp - 1) % 2][:]
                ).then_inc(s_out, 16)

            @block.gpsimd
            def _(pl):
                pl.iota(
                    iota_i[:], pattern=[[1, WN]], base=0, channel_multiplier=0
                ).then_inc(s_io, 1)

            @block.vector
            def _(v):
                v.wait_ge(s_io, 1)
                v.tensor_copy(iota_f[:], iota_i[:])
                iota_ap = iota_f[:]
                for g in range(n_grp):
                    b = g % 2
                    blob_ap = blob_sb[b][:]
                    wmat_ap = wmat_sb[b][:]
                    v.wait_ge(s_in, 16 * (g + 1))
                    v.wait_ge(s_w, g + 1)
                    for w4 in range(GRP):
                        for t in range(T):
                            gt = (g * GRP + w4) * T + t
                            if gt >= 2:
                                v.wait_ge(s_mm, gt - 1)  # a_sb[gt%2] consumed
                            A = a_sb[gt % 2]
                            # out[e, n*K+k] = (iota[n]==lid[e]) * w[e,k]
                            v.scalar_tensor_tensor(
                                _bcast_ap(A[:], 0, [A[:].ap[0], [K, WN], [1, K]]),
                                _bcast_ap(
                                    iota_ap, 0, [iota_ap.ap[0], [1, WN], [0, K]]
                                ),
                                blob_sb[b][:, w4, T * D + T + t : T * D + T + t + 1],
                                _bcast_ap(
                                    wmat_ap,
                                    w4 * T + t,
                                    [wmat_ap.ap[0], [0, WN], [GRP * T, K]],
                                ),
                                mybir.AluOpType.is_equal,
                                mybir.AluOpType.mult,
                            ).then_inc(s_a, 1)

            @block.scalar
            def _(sc):
                def evac(vg):
                    sc.wait_ge(s_mm, (vg + 1) * MM_G)
                    if vg >= 2:
                        sc.wait_ge(s_out, 16 * (vg - 1))  # stage_sb[vg%2] drained
                    sc.copy(out=stage_sb[vg % 2][:], in_=ps[vg % 2][:]).then_inc(
                        s_ev, 1
                    )

                for g in range(n_grp):
                    b = g % 2
                    sc.wait_ge(s_in, 16 * (g + 1))
                    if g >= 2:
                        sc.wait_ge(s_a, (g - 1) * MM_G)  # wmat_sb[b] readers done
                    dt_view = blob_sb[b][:, :, T * D : T * D + T]  # [128, GRP, T]
                    for k in range(K):
                        ins = sc.activation(
                            out=wmat_sb[b][:, k, :, :],
                            in_=dt_view,
                            func=Exp,
                            scale=float(inv_lambs[k]),
                        )
                    ins.then_inc(s_w, 1)
                    if g >= 1:
                        evac(g - 1)
                evac(n_grp - 1)

            @block.tensor
            def _(te):
                for g in range(n_grp):
                    b = g % 2
                    if g >= 2:
                        te.wait_ge(s_ev, g - 1)  # ps[b] evacuated
                    te.wait_ge(s_in, 16 * (g + 1))  # rhs m tiles loaded
                    for w4 in range(GRP):
                        for t in range(T):
                            gt = (g * GRP + w4) * T + t
                            te.wait_ge(s_a, gt + 1)
                            te.matmul(
                                ps[b][:, w4, :],
                                a_sb[gt % 2][:],
                                blob_sb[b][:, w4, t * D : (t + 1) * D],
                                start=(t == 0),
                                stop=(t == T - 1),
                            ).then_inc(s_mm, 1)

    return nc


def kernel(messages, timestamps, seg_ids, lambs):
    messages = np.ascontiguousarray(np.asarray(messages, dtype=np.float32))
    ts = np.asarray(timestamps, dtype=np.float32)
    seg = np.asarray(seg_ids).astype(np.int64)
    lambs_np = np.asarray(lambs, dtype=np.float32)
    n_win_total = N_NODES // WN  # 2048
    win_per_core = n_win_total // N_CORES  # 256
    n_grp = win_per_core // GRP  # 64

    counts = np.bincount(seg, minlength=N_NODES)
    ends = np.cumsum(counts)
    starts = ends - counts
    last_idx = np.maximum(ends - 1, 0)
    t_last = np.where(counts > 0, ts[last_idx], -np.inf).astype(np.float32)

    # window event ranges (contiguous because seg is sorted)
    win_start = starts[0::WN]
    win_cnt = ends[WN - 1 :: WN] - win_start
    C = int(max(128, -(-int(win_cnt.max()) // 128) * 128))
    T = C // 128
    R = T * D + 2 * T

    ar = np.arange(C)
    idx = win_start[:, None] + ar[None, :]
    valid = ar[None, :] < win_cnt[:, None]
    idxc = np.where(valid, idx, 0)

    tl_fill = np.where(counts > 0, ts[last_idx], 0.0).astype(np.float32)
    seg_g = seg[idxc]
    bases = (np.arange(n_win_total, dtype=np.int64) * WN)[:, None]
    dt_h = np.where(valid, ts[idxc] - tl_fill[seg_g], 0.0).astype(np.float32)
    lid_h = np.where(valid, seg_g - bases, 0).astype(np.float32)

    # partition-major event layout: window slot j -> (p = j // T, t = j % T)
    blob_all = np.empty((n_win_total, 128, R), np.float32)
    m_part = messages[idxc]  # [2048, C, D]
    m_part[~valid] = 0.0
    blob_all[:, :, : T * D] = m_part.reshape(n_win_total, 128, T * D)
    blob_all[:, :, T * D : T * D + T] = dt_h.reshape(n_win_total, 128, T)
    blob_all[:, :, T * D + T :] = lid_h.reshape(n_win_total, 128, T)
    del m_part
    # [core, group, partition, w4, R]
    blob_all = np.ascontiguousarray(
        blob_all.reshape(N_CORES, n_grp, GRP, 128, R).transpose(0, 1, 3, 2, 4)
    )

    inv_l = (1.0 / lambs_np.astype(np.float64)).astype(np.float32)
    nc = _build_program(n_grp, T, inv_l)

    in_maps = [{"blob": blob_all[c]} for c in range(N_CORES)]

    trace = os.environ.get("KERNEL_TRACE", "0") == "1"
    kwargs = {}
    if trace:
        kwargs = dict(trace=True, trace_cores=[0])
    res = run_bass_kernel_spmd(nc, in_maps, core_ids=list(range(N_CORES)), **kwargs)

    last_run_info.clear()
    last_run_info.update(
        exec_time_ns=res.exec_time_ns,
        mean_exec_time_ns=res.mean_exec_time_ns,
        trace=res.instructions_and_trace[1] if res.instructions_and_trace else None,
    )

    n_loc = N_NODES // N_CORES
    agg = np.empty((N_NODES, K, D), np.float32)
    for c in range(N_CORES):
        agg[c * n_loc : (c + 1) * n_loc] = res.results[c]["out_dev"].reshape(
            n_loc, K, D
        )
    return agg, t_last


# revision 16
# speedup vs baseline: 1.6204x; 1.1686x over previous
"""Trainium2 Bass kernel for ExpLambsMessageAggregator (gnn message passing).

Computes, per node n:
    t_last[n]   = max over events of node n of timestamp           (segment max)
    agg[n,k,d]  = sum_e m[e,d] * exp((t_e - t_last[n]) / lamb_k)   (segment sum)

Sharding: seg_ids are sorted, so the 65536 nodes are split into 8 ranges of
8192 nodes (one per NeuronCore); each core's events form a contiguous slice
of the event stream.  Within a core, nodes are grouped into 256 windows of
32 nodes; each window's events (contiguous) are padded to a common capacity
C (multiple of 128) so all cores run one identical SPMD program.  Windows
are processed in groups of 4 to amortize DMA-issue and ScalarE overheads.

Device algorithm per 128-event tile of a window (events laid out
partition-major: partition p holds the window's events [p*T, (p+1)*T)):
    A[e, k*32+n] = (iota[n] == local_id[e]) * exp(dt[e] / lamb_k)
        ONE fused DVE scalar_tensor_tensor op (is_equal, mult) using
        stride-0 broadcast access patterns for iota (k-repeat) and the
        per-(e,k) weights (n-broadcast)
    psum[k*32+n, d] += sum_e A[e, k*32+n] * m[e, d]
        one fp32 matmul (lhsT = A, rhs = m tile), accumulating the
        window's T tiles into a quarter PSUM bank; 4 windows share a bank.
ScalarE computes the exp weights (scale=1/lamb fused) and evacuates each
group's [128,512] PSUM bank; the (k*32+n)-partition layout is scattered to
the [n,k,d] output by the store DMA's access pattern, so the host does no
reordering.  t_last is a pure gather (timestamps are globally sorted, so
each segment's last event holds its max); empty segments get -inf to match
jax.ops.segment_max's identity.

Raw bass with explicit semaphores: this toolchain's walrus allows at most
ONE semaphore wait attached to a compute instruction, so every cross-engine
dependency is a standalone wait_ge sequencer instruction, and semaphore
counts are arranged so each instruction needs at most one fresh wait.
"""

import os
import sys
from contextlib import ExitStack

import numpy as np

if "/opt/trn_rl_repo" not in sys.path:
    sys.path.insert(0, "/opt/trn_rl_repo")

import concourse.bass as bass
from concourse import mybir
from concourse.bass_utils import run_bass_kernel_spmd

N_NODES = 65536
N_CORES = 8
D = 128
K = 4
WN = 32  # nodes per window; K*WN = 128 one-hot columns (k,n)-packed
GRP = 4  # windows per group (share one PSUM bank / blob DMA / exp batch)
F32 = mybir.dt.float32
I32 = mybir.dt.int32

# Filled in by kernel() on each call; test.py reads these for reporting.
last_run_info = {}


def _bcast_ap(base, offset_elems, dims):
    """Hand-built AP on `base` (an AP) with explicit [step, count] dims."""
    return bass.AP(tensor=base.tensor, offset=base.offset + offset_elems, ap=dims)


def _build_program(n_grp, T, inv_lambs, mdt=None):
    """One core's program: n_grp groups x GRP windows x T tiles of 128 events."""
    if mdt is None:
        mdt = mybir.dt.float32r
    R = T * D + 2 * T  # per-partition row: T m-vectors + T dt + T lid
    MM_G = GRP * T  # matmuls (= tiles) per group
    nc = bass.Bass()
    blob = nc.dram_tensor("blob", [n_grp, 128, GRP, R], mdt, kind="ExternalInput")
    out = nc.dram_tensor("out_dev", [n_grp * GRP * WN, K * D], F32, kind="ExternalOutput")
    Exp = mybir.ActivationFunctionType.Exp

    with ExitStack() as ctx:
        ecm = ctx.enter_context
        blob_sb = [ecm(nc.sbuf_tensor(f"blob_sb{i}", [128, GRP, R], mdt)) for i in range(2)]
        wmat_sb = [ecm(nc.sbuf_tensor(f"wmat_sb{i}", [128, GRP, T, K], F32)) for i in range(2)]
        a_sb = [ecm(nc.sbuf_tensor(f"a_sb{i}", [128, K * WN], mdt)) for i in range(2)]
        stage_sb = [ecm(nc.sbuf_tensor(f"stage_sb{i}", [128, GRP, D], F32)) for i in range(2)]
        ps = [ecm(nc.psum_tensor(f"ps{i}", [128, GRP, D], F32)) for i in range(2)]
        iota_i = ecm(nc.sbuf_tensor("iota_i", [128, WN], I32))
        iota_f = ecm(nc.sbuf_tensor("iota_f", [128, WN], F32))

        s_in = ecm(nc.semaphore("s_in"))  # blob DMA done: 16*(g+1)
        s_w = ecm(nc.semaphore("s_w"))  # wmat ready: g+1
        s_a = ecm(nc.semaphore("s_a"))  # A ready: gt+1
        s_mm = ecm(nc.semaphore("s_mm"))  # matmul done: gt+1
        s_ev = ecm(nc.semaphore("s_ev"))  # evac done: g+1
        s_out = ecm(nc.semaphore("s_out"))  # out DMA done: 16*(g+1)
        s_io = ecm(nc.semaphore("s_io"))  # iota ready

        def out_dst_ap(g):
            # stage partition p = n*K + k maps to out row (g*GRP+w4)*WN + n,
            # col k*D + d: dst element offset = p*D + w4*WN*K*D + d (linear
            # in p because columns are (n,k)-packed).
            base = out[:]
            return _bcast_ap(
                base,
                g * GRP * WN * (K * D),
                [[D, WN * K], [WN * K * D, GRP], [1, D]],
            )

        with nc.Block() as block:

            @block.sync
            def _(sp):
                for g in range(n_grp):
                    b = g % 2
                    if g >= 2:
                        # blob_sb[b] reuse: group g-2 matmuls done implies all
                        # its readers (PE rhs, DVE lid, ACT dt) are done.
                        sp.wait_ge(s_mm, (g - 1) * MM_G)
                    sp.dma_start(out=blob_sb[b][:], in_=blob[g][:]).then_inc(s_in, 16)
                    if g >= 1:
                        sp.wait_ge(s_ev, g)
                        sp.dma_start(
                            out=out_dst_ap(g - 1), in_=stage_sb[(g - 1) % 2][:]
                        ).then_inc(s_out, 16)
                sp.wait_ge(s_ev, n_grp)
                sp.dma_start(
                    out=out_dst_ap(n_grp - 1), in_=stage_sb[(n_grp - 1) % 2][:]
                ).then_inc(s_out, 16)

            @block.gpsimd
            def _(pl):
                pl.iota(
                    iota_i[:], pattern=[[1, WN]], base=0, channel_multiplier=0
                ).then_inc(s_io, 1)

            @block.vector
            def _(v):
                v.wait_ge(s_io, 1)
                v.tensor_copy(iota_f[:], iota_i[:])
                iota_ap = iota_f[:]
                for g in range(n_grp):
                    b = g % 2
                    blob_ap = blob_sb[b][:]
                    wmat_ap = wmat_sb[b][:]
                    v.wait_ge(s_in, 16 * (g + 1))
                    v.wait_ge(s_w, g + 1)
                    for w4 in range(GRP):
                        for t in range(T):
                            gt = (g * GRP + w4) * T + t
                            if gt >= 2:
                                v.wait_ge(s_mm, gt - 1)  # a_sb[gt%2] consumed
                            A = a_sb[gt % 2]
                            # out[e, n*K+k] = (iota[n]==lid[e]) * w[e,k]
                            v.scalar_tensor_tensor(
                                _bcast_ap(A[:], 0, [A[:].ap[0], [K, WN], [1, K]]),
                                _bcast_ap(
                                    iota_ap, 0, [iota_ap.ap[0], [1, WN], [0, K]]
                                ),
                                blob_sb[b][
                                    :, w4, T * D + T + t : T * D + T + t + 1
                                ].bitcast(F32),
                                _bcast_ap(
                                    wmat_ap,
                                    (w4 * T + t) * K,
                                    [wmat_ap.ap[0], [0, WN], [1, K]],
                                ),
                                mybir.AluOpType.is_equal,
                                mybir.AluOpType.mult,
                            ).then_inc(s_a, 1)

            @block.scalar
            def _(sc):
                def evac(vg):
                    sc.wait_ge(s_mm, (vg + 1) * MM_G)
                    if vg >= 2:
                        sc.wait_ge(s_out, 16 * (vg - 1))  # stage_sb[vg%2] drained
                    sc.copy(out=stage_sb[vg % 2][:], in_=ps[vg % 2][:]).then_inc(
                        s_ev, 1
                    )

                for g in range(n_grp):
                    b = g % 2
                    sc.wait_ge(s_in, 16 * (g + 1))
                    if g >= 2:
                        sc.wait_ge(s_a, (g - 1) * MM_G)  # wmat_sb[b] readers done
                    dt_view = blob_sb[b][:, :, T * D : T * D + T].bitcast(F32)
                    for k in range(K):
                        ins = sc.activation(
                            out=wmat_sb[b][:, :, :, k],
                            in_=dt_view,
                            func=Exp,
                            scale=float(inv_lambs[k]),
                        )
                    ins.then_inc(s_w, 1)
                    if g >= 1:
                        evac(g - 1)
                evac(n_grp - 1)

            @block.tensor
            def _(te):
                for g in range(n_grp):
                    b = g % 2
                    if g >= 2:
                        te.wait_ge(s_ev, g - 1)  # ps[b] evacuated
                    te.wait_ge(s_in, 16 * (g + 1))  # rhs m tiles loaded
                    for w4 in range(GRP):
                        for t in range(T):
                            gt = (g * GRP + w4) * T + t
                            te.wait_ge(s_a, gt + 1)
                            te.matmul(
                                ps[b][:, w4, :],
                                a_sb[gt % 2][:],
                                blob_sb[b][:, w4, t * D : (t + 1) * D],
                                start=(t == 0),
                                stop=(t == T - 1),
                            ).then_inc(s_mm, 1)

    return nc


def kernel(messages, timestamps, seg_ids, lambs):
    messages = np.ascontiguousarray(np.asarray(messages, dtype=np.float32))
    ts = np.asarray(timestamps, dtype=np.float32)
    seg = np.asarray(seg_ids).astype(np.int64)
    lambs_np = np.asarray(lambs, dtype=np.float32)
    n_win_total = N_NODES // WN  # 2048
    win_per_core = n_win_total // N_CORES  # 256
    n_grp = win_per_core // GRP  # 64

    counts = np.bincount(seg, minlength=N_NODES)
    ends = np.cumsum(counts)
    starts = ends - counts
    last_idx = np.maximum(ends - 1, 0)
    t_last = np.where(counts > 0, ts[last_idx], -np.inf).astype(np.float32)

    # window event ranges (contiguous because seg is sorted)
    win_start = starts[0::WN]
    win_cnt = ends[WN - 1 :: WN] - win_start
    C = int(max(128, -(-int(win_cnt.max()) // 128) * 128))
    T = C // 128
    R = T * D + 2 * T

    ar = np.arange(C)
    idx = win_start[:, None] + ar[None, :]
    valid = ar[None, :] < win_cnt[:, None]
    idxc = np.where(valid, idx, 0)

    tl_fill = np.where(counts > 0, ts[last_idx], 0.0).astype(np.float32)
    seg_g = seg[idxc]
    bases = (np.arange(n_win_total, dtype=np.int64) * WN)[:, None]
    dt_h = np.where(valid, ts[idxc] - tl_fill[seg_g], 0.0).astype(np.float32)
    lid_h = np.where(valid, seg_g - bases, 0).astype(np.float32)

    # partition-major event layout: window slot j -> (p = j // T, t = j % T)
    blob_all = np.empty((n_win_total, 128, R), np.float32)
    m_part = messages[idxc]  # [2048, C, D]
    m_part[~valid] = 0.0
    blob_all[:, :, : T * D] = m_part.reshape(n_win_total, 128, T * D)
    blob_all[:, :, T * D : T * D + T] = dt_h.reshape(n_win_total, 128, T)
    blob_all[:, :, T * D + T :] = lid_h.reshape(n_win_total, 128, T)
    del m_part
    # [core, group, partition, w4, R]
    blob_all = np.ascontiguousarray(
        blob_all.reshape(N_CORES, n_grp, GRP, 128, R).transpose(0, 1, 3, 2, 4)
    )

    inv_l = (1.0 / lambs_np.astype(np.float64)).astype(np.float32)
    mdt = F32 if os.environ.get("KERNEL_FP32", "0") == "1" else mybir.dt.float32r
    nc = _build_program(n_grp, T, inv_l, mdt=mdt)

    in_maps = [{"blob": blob_all[c]} for c in range(N_CORES)]

    trace = os.environ.get("KERNEL_TRACE", "0") == "1"
    kwargs = {}
    if trace:
        kwargs = dict(trace=True, trace_cores=[0])
    res = run_bass_kernel_spmd(nc, in_maps, core_ids=list(range(N_CORES)), **kwargs)

    last_run_info.clear()
    last_run_info.update(
        exec_time_ns=res.exec_time_ns,
        mean_exec_time_ns=res.mean_exec_time_ns,
        trace=res.instructions_and_trace[1] if res.instructions_and_trace else None,
    )

    n_loc = N_NODES // N_CORES
    agg = np.empty((N_NODES, K, D), np.float32)
    for c in range(N_CORES):
        agg[c * n_loc : (c + 1) * n_loc] = res.results[c]["out_dev"].reshape(
            n_loc, K, D
        )
    return agg, t_last


# revision 17
# speedup vs baseline: 1.6394x; 1.0117x over previous
"""Trainium2 Bass kernel for ExpLambsMessageAggregator (gnn message passing).

Computes, per node n:
    t_last[n]   = max over events of node n of timestamp           (segment max)
    agg[n,k,d]  = sum_e m[e,d] * exp((t_e - t_last[n]) / lamb_k)   (segment sum)

Sharding: seg_ids are sorted, so the 65536 nodes are split into 8 ranges of
8192 nodes (one per NeuronCore); each core's events form a contiguous slice
of the event stream.  Within a core, nodes are grouped into 256 windows of
32 nodes; each window's events (contiguous) are padded to a common capacity
C (multiple of 128) so all cores run one identical SPMD program.  Windows
are processed in groups of 4 to amortize DMA-issue and ScalarE overheads.

Device algorithm per 128-event tile of a window (events laid out
partition-major: partition p holds the window's events [p*T, (p+1)*T)):
    A[e, k*32+n] = (iota[n] == local_id[e]) * exp(dt[e] / lamb_k)
        ONE fused DVE scalar_tensor_tensor op (is_equal, mult) using
        stride-0 broadcast access patterns for iota (k-repeat) and the
        per-(e,k) weights (n-broadcast)
    psum[k*32+n, d] += sum_e A[e, k*32+n] * m[e, d]
        one fp32 matmul (lhsT = A, rhs = m tile), accumulating the
        window's T tiles into a quarter PSUM bank; 4 windows share a bank.
ScalarE computes the exp weights (scale=1/lamb fused) and evacuates each
group's [128,512] PSUM bank; the (k*32+n)-partition layout is scattered to
the [n,k,d] output by the store DMA's access pattern, so the host does no
reordering.  t_last is a pure gather (timestamps are globally sorted, so
each segment's last event holds its max); empty segments get -inf to match
jax.ops.segment_max's identity.

Raw bass with explicit semaphores: this toolchain's walrus allows at most
ONE semaphore wait attached to a compute instruction, so every cross-engine
dependency is a standalone wait_ge sequencer instruction, and semaphore
counts are arranged so each instruction needs at most one fresh wait.
"""

import os
import sys
from contextlib import ExitStack

import numpy as np

if "/opt/trn_rl_repo" not in sys.path:
    sys.path.insert(0, "/opt/trn_rl_repo")

import concourse.bass as bass
from concourse import mybir
from concourse.bass_utils import run_bass_kernel_spmd

N_NODES = 65536
N_CORES = 8
D = 128
K = 4
WN = 32  # nodes per window; K*WN = 128 one-hot columns (k,n)-packed
GRP = 4  # windows per group (share one PSUM bank / blob DMA / exp batch)
F32 = mybir.dt.float32
I32 = mybir.dt.int32

# Filled in by kernel() on each call; test.py reads these for reporting.
last_run_info = {}


def _bcast_ap(base, offset_elems, dims):
    """Hand-built AP on `base` (an AP) with explicit [step, count] dims."""
    return bass.AP(tensor=base.tensor, offset=base.offset + offset_elems, ap=dims)


def _build_program(n_grp, T, inv_lambs, mdt=None):
    """One core's program: n_grp groups x GRP windows x T tiles of 128 events."""
    if mdt is None:
        mdt = mybir.dt.float32r
    R = T * D + 2 * T  # per-partition row: T m-vectors + T dt + T lid
    MM_G = GRP * T  # matmuls (= tiles) per group
    nc = bass.Bass()
    blob = nc.dram_tensor("blob", [n_grp, 128, GRP, R], mdt, kind="ExternalInput")
    out = nc.dram_tensor("out_dev", [n_grp * GRP * WN, K * D], F32, kind="ExternalOutput")
    Exp = mybir.ActivationFunctionType.Exp

    with ExitStack() as ctx:
        ecm = ctx.enter_context
        blob_sb = [ecm(nc.sbuf_tensor(f"blob_sb{i}", [128, GRP, R], mdt)) for i in range(2)]
        wmat_sb = [ecm(nc.sbuf_tensor(f"wmat_sb{i}", [128, GRP, T, K], F32)) for i in range(2)]
        a_sb = [ecm(nc.sbuf_tensor(f"a_sb{i}", [128, K * WN], mdt)) for i in range(4)]
        stage_sb = [ecm(nc.sbuf_tensor(f"stage_sb{i}", [128, GRP, D], F32)) for i in range(2)]
        ps = [ecm(nc.psum_tensor(f"ps{i}", [128, GRP, D], F32)) for i in range(2)]
        iota_i = ecm(nc.sbuf_tensor("iota_i", [128, WN], I32))
        iota_f = ecm(nc.sbuf_tensor("iota_f", [128, WN], F32))

        s_in = ecm(nc.semaphore("s_in"))  # blob DMA done: 16*(g+1)
        s_w = ecm(nc.semaphore("s_w"))  # wmat ready: g+1
        s_a = ecm(nc.semaphore("s_a"))  # A ready: gt+1
        s_mm = ecm(nc.semaphore("s_mm"))  # matmul done: gt+1
        s_ev = ecm(nc.semaphore("s_ev"))  # evac done: g+1
        s_out = ecm(nc.semaphore("s_out"))  # out DMA done: 16*(g+1)
        s_io = ecm(nc.semaphore("s_io"))  # iota ready

        def out_dst_ap(g):
            # stage partition p = n*K + k maps to out row (g*GRP+w4)*WN + n,
            # col k*D + d: dst element offset = p*D + w4*WN*K*D + d (linear
            # in p because columns are (n,k)-packed).
            base = out[:]
            return _bcast_ap(
                base,
                g * GRP * WN * (K * D),
                [[D, WN * K], [WN * K * D, GRP], [1, D]],
            )

        with nc.Block() as block:

            @block.sync
            def _(sp):
                for g in range(n_grp):
                    b = g % 2
                    if g >= 2:
                        # blob_sb[b] reuse: group g-2 matmuls done implies all
                        # its readers (PE rhs, DVE lid, ACT dt) are done.
                        sp.wait_ge(s_mm, (g - 1) * MM_G)
                    sp.dma_start(out=blob_sb[b][:], in_=blob[g][:]).then_inc(s_in, 16)
                    if g >= 1:
                        sp.wait_ge(s_ev, g)
                        sp.dma_start(
                            out=out_dst_ap(g - 1), in_=stage_sb[(g - 1) % 2][:]
                        ).then_inc(s_out, 16)
                sp.wait_ge(s_ev, n_grp)
                sp.dma_start(
                    out=out_dst_ap(n_grp - 1), in_=stage_sb[(n_grp - 1) % 2][:]
                ).then_inc(s_out, 16)

            @block.gpsimd
            def _(pl):
                pl.iota(
                    iota_i[:], pattern=[[1, WN]], base=0, channel_multiplier=0
                ).then_inc(s_io, 1)

            @block.vector
            def _(v):
                v.wait_ge(s_io, 1)
                v.tensor_copy(iota_f[:], iota_i[:])
                iota_ap = iota_f[:]
                for g in range(n_grp):
                    b = g % 2
                    blob_ap = blob_sb[b][:]
                    wmat_ap = wmat_sb[b][:]
                    v.wait_ge(s_in, 16 * (g + 1))
                    v.wait_ge(s_w, g + 1)
                    for w4 in range(GRP):
                        base = (g * GRP + w4) * T
                        # a_sb ring (4 deep): one batched WAR wait per window
                        if base + T - 4 > 0:
                            v.wait_ge(s_mm, base + T - 4)
                        for t in range(T):
                            gt = base + t
                            A = a_sb[gt % 4]
                            # out[e, n*K+k] = (iota[n]==lid[e]) * w[e,k]
                            v.scalar_tensor_tensor(
                                _bcast_ap(A[:], 0, [A[:].ap[0], [K, WN], [1, K]]),
                                _bcast_ap(
                                    iota_ap, 0, [iota_ap.ap[0], [1, WN], [0, K]]
                                ),
                                blob_sb[b][
                                    :, w4, T * D + T + t : T * D + T + t + 1
                                ].bitcast(F32),
                                _bcast_ap(
                                    wmat_ap,
                                    (w4 * T + t) * K,
                                    [wmat_ap.ap[0], [0, WN], [1, K]],
                                ),
                                mybir.AluOpType.is_equal,
                                mybir.AluOpType.mult,
                            ).then_inc(s_a, 1)

            @block.scalar
            def _(sc):
                def evac(vg):
                    sc.wait_ge(s_mm, (vg + 1) * MM_G)
                    if vg >= 2:
                        sc.wait_ge(s_out, 16 * (vg - 1))  # stage_sb[vg%2] drained
                    sc.copy(out=stage_sb[vg % 2][:], in_=ps[vg % 2][:]).then_inc(
                        s_ev, 1
                    )

                for g in range(n_grp):
                    b = g % 2
                    sc.wait_ge(s_in, 16 * (g + 1))
                    if g >= 2:
                        sc.wait_ge(s_a, (g - 1) * MM_G)  # wmat_sb[b] readers done
                    dt_view = blob_sb[b][:, :, T * D : T * D + T].bitcast(F32)
                    for k in range(K):
                        ins = sc.activation(
                            out=wmat_sb[b][:, :, :, k],
                            in_=dt_view,
                            func=Exp,
                            scale=float(inv_lambs[k]),
                        )
                    ins.then_inc(s_w, 1)
                    if g >= 1:
                        evac(g - 1)
                evac(n_grp - 1)

            @block.tensor
            def _(te):
                for g in range(n_grp):
                    b = g % 2
                    if g >= 2:
                        te.wait_ge(s_ev, g - 1)  # ps[b] evacuated
                    te.wait_ge(s_in, 16 * (g + 1))  # rhs m tiles loaded
                    for w4 in range(GRP):
                        for t in range(T):
                            gt = (g * GRP + w4) * T + t
                            te.wait_ge(s_a, gt + 1)
                            te.matmul(
                                ps[b][:, w4, :],
                                a_sb[gt % 4][:],
                                blob_sb[b][:, w4, t * D : (t + 1) * D],
                                start=(t == 0),
                                stop=(t == T - 1),
                            ).then_inc(s_mm, 1)

    return nc


def kernel(messages, timestamps, seg_ids, lambs):
    messages = np.ascontiguousarray(np.asarray(messages, dtype=np.float32))
    ts = np.asarray(timestamps, dtype=np.float32)
    seg = np.asarray(seg_ids).astype(np.int64)
    lambs_np = np.asarray(lambs, dtype=np.float32)
    n_win_total = N_NODES // WN  # 2048
    win_per_core = n_win_total // N_CORES  # 256
    n_grp = win_per_core // GRP  # 64

    counts = np.bincount(seg, minlength=N_NODES)
    ends = np.cumsum(counts)
    starts = ends - counts
    last_idx = np.maximum(ends - 1, 0)
    t_last = np.where(counts > 0, ts[last_idx], -np.inf).astype(np.float32)

    # window event ranges (contiguous because seg is sorted)
    win_start = starts[0::WN]
    win_cnt = ends[WN - 1 :: WN] - win_start
    C = int(max(128, -(-int(win_cnt.max()) // 128) * 128))
    T = C // 128
    R = T * D + 2 * T

    ar = np.arange(C)
    idx = win_start[:, None] + ar[None, :]
    valid = ar[None, :] < win_cnt[:, None]
    idxc = np.where(valid, idx, 0)

    tl_fill = np.where(counts > 0, ts[last_idx], 0.0).astype(np.float32)
    seg_g = seg[idxc]
    bases = (np.arange(n_win_total, dtype=np.int64) * WN)[:, None]
    dt_h = np.where(valid, ts[idxc] - tl_fill[seg_g], 0.0).astype(np.float32)
    lid_h = np.where(valid, seg_g - bases, 0).astype(np.float32)

    # partition-major event layout: window slot j -> (p = j // T, t = j % T)
    blob_all = np.empty((n_win_total, 128, R), np.float32)
    m_part = messages[idxc]  # [2048, C, D]
    m_part[~valid] = 0.0
    blob_all[:, :, : T * D] = m_part.reshape(n_win_total, 128, T * D)
    blob_all[:, :, T * D : T * D + T] = dt_h.reshape(n_win_total, 128, T)
    blob_all[:, :, T * D + T :] = lid_h.reshape(n_win_total, 128, T)
    del m_part
    # [core, group, partition, w4, R]
    blob_all = np.ascontiguousarray(
        blob_all.reshape(N_CORES, n_grp, GRP, 128, R).transpose(0, 1, 3, 2, 4)
    )

    inv_l = (1.0 / lambs_np.astype(np.float64)).astype(np.float32)
    mdt = F32 if os.environ.get("KERNEL_FP32", "0") == "1" else mybir.dt.float32r
    nc = _build_program(n_grp, T, inv_l, mdt=mdt)

    in_maps = [{"blob": blob_all[c]} for c in range(N_CORES)]

    trace = os.environ.get("KERNEL_TRACE", "0") == "1"
    kwargs = {}
    if trace:
        kwargs = dict(trace=True, trace_cores=[0])
    res = run_bass_kernel_spmd(nc, in_maps, core_ids=list(range(N_CORES)), **kwargs)

    last_run_info.clear()
    last_run_info.update(
        exec_time_ns=res.exec_time_ns,
        mean_exec_time_ns=res.mean_exec_time_ns,
        trace=res.instructions_and_trace[1] if res.instructions_and_trace else None,
    )

    n_loc = N_NODES // N_CORES
    agg = np.empty((N_NODES, K, D), np.float32)
    for c in range(N_CORES):
        agg[c * n_loc : (c + 1) * n_loc] = res.results[c]["out_dev"].reshape(
            n_loc, K, D
        )
    return agg, t_last
